# revision 35
# baseline (speedup 1.0000x reference)
# Trainium2 Bass kernel for nn_CVXPolicy_MultiQuadcopter.
#
# Math (per sample):
#   x  = concat([t, z]);  h1 = tanh(x W1 + b1);  h2 = tanh(h1 W2 + b2)
#   p  = h2 W3 + b3;  c = S(p)  (per-agent sparse linear map, 3072->1024)
#   s  = ||c||^2 ; w = W(256*s) ; u* = -sqrt(256*w/s) * c
#
# Host-side folds:
#   - S is linear: c = h2a @ W3a with h2a = [h2; 1], W3a = [[W3 S],[b3 S]].
#   - s = h2a^T (W3a W3a^T) h2a = h2a^T Ga h2a with the 101x101 Gram
#     matrix Ga precomputed on host.  No elementwise squares of c on
#     device, and k is ready BEFORE mm3 so the -k scale fuses with the
#     PSUM->SBUF move.
#   - z is bf16-cast and transposed to [D, B] on host (contraction on
#     partitions: no on-chip transposes, half the HBM traffic).  DRAM
#     layout gives each partition a contiguous run per DMA so HWDGE
#     descriptors are 8KB (descriptor GENERATION ~5ns/desc is the
#     DMA-issue bottleneck, not bandwidth).
#
# Lambert-W needs no iteration for k's accuracy (dk/k = dw/2w ~ dw/20):
#   w = alpha*ln(256 s) + beta  with ln from the fp32-exponent bit trick
#   k = 16*sqrt(w/s): rsqrt seed via ACT exp of the same bit trick,
#   one Newton step on DVE.  Only {tanh, exp} ACT functions are used ->
#   a single ACT_TABLE_LOAD for the whole kernel.
#
# Engine layout: z loads on sync/HWDGE; weight loads + output stores on
# the otherwise-idle GpSimd SWDGE ring; dummy matmuls keep the PE's HAM
# clock gate warm across the DMA-wait and Lambert windows.
#
# Sharding: pure data parallelism, batch 8192 -> 8 shards of 1024 rows.
# Output is written bf16 (within tolerance) and upcast on host.

import numpy as np
import ml_dtypes
from contextlib import ExitStack

import concourse.bass as bass
import concourse.tile as tile
from concourse import bacc, mybir
from concourse.bass_utils import run_bass_kernel_spmd

F32 = mybir.dt.float32
I32 = mybir.dt.int32
BF16 = mybir.dt.bfloat16

N_CORES = 8
BATCH = 8192
B = BATCH // N_CORES      # 1024 batch rows per core
D = 3072                  # state dim
H = 100                   # hidden
HA = H + 1                # hidden + ones row
CD = 1024                 # control dim
NCH = D // 128            # 24 contraction chunks for mm1
# batch groups: a big streaming group and a small final group so the
# post-stream serial tail (tanh->mm2->tanh->gram->s->lambert->mm3) covers
# as few columns as possible
GROUPS = [(0, 512), (512, 256), (768, 256)]   # (col start, width)
NG = len(GROUPS)
ZB = 8                    # j-chunks per z DMA
MASS = 0.5

AF = mybir.ActivationFunctionType
ALU = mybir.AluOpType

LN2 = 0.6931471805599453
LN256 = 5.545177444479562
# ln(x) ~= LN2 * (float(bitcast_i32(x)) * 2^-23 - 126.94269504)
LNA = LN2 / (1 << 23)
LNB = -126.94269504 * LN2
# w0 = alpha*ln(256 s) + beta (fit of W(e^L)=L-lnL+lnL/L over L in [11,14])
W0_ALPHA = 0.9103
W0_BETA = -1.2024
W0_A = W0_ALPHA * LNA
W0_B = W0_ALPHA * (LNB + LN256) + W0_BETA
# rsqrt seed: y0 = exp(aif*(-0.5*LNA) + (-0.5*LNB)) ~= 1/sqrt(a)
RS_SCALE = -0.5 * LNA
RS_BIAS = -0.5 * LNB


def build_kernel():
    nc = bacc.Bacc(None, target_bir_lowering=False, enable_partition_id=False)

    # zt[p, cstart_g*NCH + j*Wg + f] = z[gstart_g + f, j*128 + p]
    zt_d = nc.declare_dram_parameter("zt", [128, NCH * B], BF16, isOutput=False)
    # w1x chunk 24: row0 = W1[0,:] (t weights), row1 = b1 (ones-row weights)
    w1x_d = nc.declare_dram_parameter("w1x", [128, (NCH + 1) * H], BF16, isOutput=False)
    # wall: [0:101, 0:100] = [W2; b2], [0:101, 128:229] = Ga,
    # [0:101, 256:1280] = W3a, [0:2, 1280:2304] = te2 ([t; ones]).
    # Padded to 128 partitions: DMAs with <128 partitions are NOT split
    # across the 16 SDMA engines and serialize on engine 0, straggling
    # every later z-completion semaphore by ~10us.
    wall_d = nc.declare_dram_parameter("wall", [128, 2304], BF16, isOutput=False)
    out_d = nc.declare_dram_parameter("out", [B, CD], BF16, isOutput=True)

    with ExitStack() as ctx:
        tc = ctx.enter_context(tile.TileContext(nc))

        const = ctx.enter_context(tc.tile_pool(name="const", bufs=1))
        zpool = ctx.enter_context(tc.tile_pool(name="zt", bufs=NG))
        hpool = ctx.enter_context(tc.tile_pool(name="hs", bufs=2))
        lwp = ctx.enter_context(tc.tile_pool(name="lw", bufs=1))
        opool = ctx.enter_context(tc.tile_pool(name="outs", bufs=3))
        h1_ps = ctx.enter_context(tc.tile_pool(name="h1p", bufs=1, space="PSUM"))
        hq_ps = ctx.enter_context(tc.tile_pool(name="hqp", bufs=2, space="PSUM"))
        c_ps = ctx.enter_context(tc.tile_pool(name="cp", bufs=2, space="PSUM"))
        s_ps = ctx.enter_context(tc.tile_pool(name="sp", bufs=1, space="PSUM"))

        # ---- t=0: warm the ACT table (tanh+exp set) and the PE HAM clock
        # under the DMA shadow; small on-chip constants.
        warm_in = const.tile([128, 1], F32, tag="warm_in")
        nc.vector.memset(warm_in[:], 0.0)
        warm_out = const.tile([128, 1], F32, tag="warm_out")
        nc.scalar.activation(warm_out[:], warm_in[:], AF.Tanh, bias=warm_in[:])
        wsrc = const.tile([128, 512], BF16, tag="wsrc")
        nc.vector.memset(wsrc[:], 0.0)
        wps = h1_ps.tile([128, 512], F32, tag="h1p", name="wps")
        for _ in range(8):
            nc.tensor.matmul(wps[:], wsrc[:, 0:128], wsrc[:], start=True, stop=True)
        ones_a = const.tile([HA, 1], BF16, tag="ones_a")
        nc.vector.memset(ones_a[:], 1.0)
        rsb = const.tile([128, 1], F32, tag="rsb")
        nc.vector.memset(rsb[:], RS_BIAS)

        # ---- all loads on ONE ring (sync/HWDGE), interleaved in need-order:
        # secondary rings (scalar HWDGE, gpsimd SWDGE) get starved by the
        # z stream, so weights ride the same FIFO just ahead of their use.
        w1s = const.tile([128, NCH + 1, H], BF16, tag="w1s")
        wall = const.tile([128, 2304], BF16, tag="wall")
        w2a = wall[0:HA, 0:H]
        ga = wall[0:HA, 128:128 + HA]
        te2 = wall[0:2, 1280:1280 + B]

        def load_w1x(part):
            cs, ce = [(0, 9), (9, 17), (17, 25)][part]
            nc.sync.dma_start(
                w1s[:, cs:ce, :],
                w1x_d[:, cs * H:ce * H].rearrange("p (c h) -> p c h", c=ce - cs),
            )

        zg = {g: zpool.tile([128, NCH, GROUPS[g][1]], BF16, tag=f"zg{g}",
                            name=f"zg{g}")
              for g in range(NG)}

        def load_z(g, jb):
            gs, w = GROUPS[g]
            c0 = NCH * gs + jb * ZB * w
            nc.sync.dma_start(
                zg[g][:, jb * ZB:(jb + 1) * ZB, :],
                zt_d[:, c0:c0 + ZB * w].rearrange("p (c f) -> p c f", c=ZB),
            )

        load_w1x(0)
        load_w1x(1)
        load_w1x(2)
        nc.sync.dma_start(wall[:], wall_d[:])
        for g in range(NG):
            for jb in range(NCH // ZB):
                load_z(g, jb)

        s_all = s_ps.tile([128, B // 128], F32, tag="s_all")
        SCOL = [GROUPS[g][0] // 128 for g in range(NG)]

        def lambert(g):
            """kneg = -16*sqrt(w/s) for group g's chunks of s_all."""
            cpg = GROUPS[g][1] // 128

            def lt(nm, dt=F32):
                return lwp.tile([128, cpg], dt, tag=f"{nm}{g}", name=f"{nm}{g}")

            sv = s_all[:, SCOL[g]:SCOL[g] + cpg]
            sg = lt("sg")
            nc.vector.tensor_scalar_max(sg[:], sv, 1e-20)
            sif = lt("sif")
            nc.vector.tensor_copy(sif[:], sg[:].bitcast(I32))
            w = lt("w")
            nc.vector.tensor_scalar(w[:], sif[:], W0_A, W0_B, ALU.mult, ALU.add)
            rs = lt("rs")
            nc.vector.reciprocal_approx_fast(rs[:], sg[:])
            a = lt("a")
            nc.vector.tensor_mul(a[:], w[:], rs[:])
            aif = lt("aif")
            nc.vector.tensor_copy(aif[:], a[:].bitcast(I32))
            y0 = lt("y0")
            nc.scalar.activation(y0[:], aif[:], AF.Exp, bias=rsb[:], scale=RS_SCALE)
            # Newton step with -16 folded in: kneg = a*y0*(8*a*y0^2 - 24)
            yy = lt("yy")
            nc.vector.tensor_mul(yy[:], y0[:], y0[:])
            nc.vector.tensor_mul(yy[:], a[:], yy[:])
            nc.vector.tensor_scalar(yy[:], yy[:], 8.0, -24.0, ALU.mult, ALU.add)
            nc.vector.tensor_mul(y0[:], y0[:], yy[:])
            kneg = lt("kneg")
            nc.vector.tensor_mul(kneg[:], a[:], y0[:])
            return kneg

        for g in range(NG):
            cs, W = GROUPS[g]
            cpg = W // 128
            # mm1: h1p[h, b] = W1[0,h]*t[b] + b1[h] + sum_d W1[1+d,h] zT[d,b]
            h1p = h1_ps.tile([H, W], F32, tag="h1p", name="h1p")
            nc.tensor.matmul(
                h1p[:], w1s[0:2, NCH, :], wall[0:2, 1280 + cs:1280 + cs + W],
                start=True, stop=False,
            )
            for j in range(NCH):
                nc.tensor.matmul(
                    h1p[:], w1s[:, j, :], zg[g][:, j, :],
                    start=False, stop=(j == NCH - 1),
                )
            # tanh -> h1a with ones row at partition 100 (feeds b2 via w2a)
            h1a = hpool.tile([128, W], BF16, tag="h1s", name="h1a")
            nc.vector.memset(h1a[96:128, :], 1.0)
            nc.scalar.activation(h1a[0:H, :], h1p[:], AF.Tanh, bias=warm_in[0:H, :])
            # mm2 + tanh -> h2a with ones row at partition 100
            h2p = hq_ps.tile([H, W], F32, tag="hqp", name="h2p")
            nc.tensor.matmul(h2p[:], w2a, h1a[0:HA, :], start=True, stop=True)
            h2a = hpool.tile([128, W], BF16, tag="h2a", name="h2a")
            nc.vector.memset(h2a[96:128, :], 1.0)
            nc.scalar.activation(h2a[0:H, :], h2p[:], AF.Tanh, bias=warm_in[0:H, :])
            # Gram: q = Ga @ h2a ; sel = h2a*q ; s = colsum(sel) via PE
            qp = hq_ps.tile([HA, W], F32, tag="hqp", name="qp")
            nc.tensor.matmul(qp[:], ga, h2a[0:HA, :], start=True, stop=True)
            sel = hpool.tile([HA, W], BF16, tag="sel", name="sel")
            nc.vector.tensor_mul(sel[:], h2a[0:HA, :], qp[:])
            for i in range(cpg):
                nc.tensor.matmul(
                    s_all[:, SCOL[g] + i:SCOL[g] + i + 1],
                    sel[:, i * 128:(i + 1) * 128], ones_a[:],
                    start=True, stop=True,
                )
            kneg = lambert(g)
            # mm3 per 128-chunk into a 2-bank PSUM tile; ONE fused -k scale
            # [128,1024] per chunk, whole chunks alternating DVE / ACT so
            # two chunks are in flight on different engines
            for i in range(cpg):
                ot = opool.tile([128, CD], BF16, tag="ot", name="ot")
                cp = c_ps.tile([128, CD], F32, tag="cp", name="cp")
                for hf in range(2):
                    nc.tensor.matmul(
                        cp[:, hf * 512:(hf + 1) * 512],
                        h2a[0:HA, i * 128:(i + 1) * 128],
                        wall[0:HA, 256 + hf * 512:256 + (hf + 1) * 512],
                        start=True, stop=True,
                    )
                if (cs // 128 + i) % 2 == 0:
                    nc.vector.tensor_scalar(
                        ot[:], cp[:], kneg[:, i:i + 1], None, ALU.mult,
                    )
                else:
                    nc.scalar.activation(
                        ot[:], cp[:], AF.Copy, bias=0.0, scale=kneg[:, i:i + 1],
                    )
                bt = cs // 128 + i
                # SWDGE: keeps stores off the sync HWDGE ring (descriptor-gen
                # bound) and off the scalar ring (would block later ACT ops)
                nc.gpsimd.dma_start(out_d[bt * 128:(bt + 1) * 128, :], ot[:])

    nc.compile()
    return nc


def host_prep(z, t, W1, b1, W2, b2, W3, b3):
    """Host-side weight folds, bf16 casts, z transpose, per-core shards."""
    f = np.float32
    bf = ml_dtypes.bfloat16
    z = np.asarray(z, f)
    t = np.asarray(t, f)
    W1 = np.asarray(W1, f)
    b1 = np.asarray(b1, f)
    W2 = np.asarray(W2, f)
    b2 = np.asarray(b2, f)
    W3 = np.asarray(W3, f)
    b3 = np.asarray(b3, f)

    # mm1 stationary chunks: w1x[p, j*H + h] = W1[1 + j*128 + p, h];
    # chunk NCH rows 0/1 carry W1[0,:] (t weights) and b1 (ones-row weights)
    w1x = np.zeros((128, NCH + 1, H), np.float32)
    w1x[:, :NCH, :] = W1[1:, :].reshape(NCH, 128, H).transpose(1, 0, 2)
    w1x[0, NCH, :] = W1[0, :]
    w1x[1, NCH, :] = b1
    w1x = np.ascontiguousarray(w1x.reshape(128, (NCH + 1) * H)).astype(bf)

    # fold the p -> c map into W3 / b3, then the ones-row bias fold
    W3r = W3.reshape(H, CD // 4, 12)
    W3S = np.empty((H, CD // 4, 4), f)
    W3S[..., 0] = (W3r[..., 6] + W3r[..., 7] + W3r[..., 8]) / MASS
    W3S[..., 1] = W3r[..., 9]
    W3S[..., 2] = W3r[..., 10]
    W3S[..., 3] = W3r[..., 11]
    b3r = b3.reshape(CD // 4, 12)
    b3S = np.empty((CD // 4, 4), f)
    b3S[..., 0] = (b3r[..., 6] + b3r[..., 7] + b3r[..., 8]) / MASS
    b3S[..., 1] = b3r[..., 9]
    b3S[..., 2] = b3r[..., 10]
    b3S[..., 3] = b3r[..., 11]
    w3a = np.concatenate([W3S.reshape(H, CD), b3S.reshape(1, CD)], axis=0)
    ga = w3a @ w3a.T
    wall0 = np.zeros((128, 2304), f)
    wall0[0:H, 0:H] = W2
    wall0[H, 0:H] = b2
    wall0[0:HA, 128:128 + HA] = ga
    wall0[0:HA, 256:256 + CD] = w3a
    wall0[1, 1280:1280 + B] = 1.0

    zb = z.astype(bf)

    in_maps = []
    for c in range(N_CORES):
        sl = slice(c * B, (c + 1) * B)
        # zt[p, NCH*gs + j*W + f] = z[c*B + gs + f, j*128 + p]
        zT = zb[sl].T.reshape(NCH, 128, B)
        zt = np.empty((128, NCH * B), zb.dtype)
        for gs, W in GROUPS:
            blk = zT[:, :, gs:gs + W].transpose(1, 0, 2).reshape(128, NCH * W)
            zt[:, NCH * gs:NCH * (gs + W)] = blk
        zt = np.ascontiguousarray(zt)
        wall0[0, 1280:1280 + B] = t[sl, 0]
        in_maps.append({
            "zt": zt,
            "w1x": w1x,
            "wall": np.ascontiguousarray(wall0).astype(bf),
        })
    return in_maps


_NC_CACHE = None


def _get_nc():
    global _NC_CACHE
    if _NC_CACHE is None:
        _NC_CACHE = build_kernel()
    return _NC_CACHE


def run(inputs, trace=False):
    """Returns (full_output, BassKernelResults)."""
    nc = _get_nc()
    in_maps = host_prep(**inputs)
    res = run_bass_kernel_spmd(
        nc, in_maps, list(range(N_CORES)), trace=trace,
    )
    out = np.concatenate(
        [np.asarray(r["out"]).astype(np.float32) for r in res.results], axis=0
    )
    return out, res


def kernel(**inputs):
    out, _ = run(inputs)
    return out


# revision 36
# speedup vs baseline: 1.0030x; 1.0030x over previous
# Trainium2 Bass kernel for nn_CVXPolicy_MultiQuadcopter.
#
# Math (per sample):
#   x  = concat([t, z]);  h1 = tanh(x W1 + b1);  h2 = tanh(h1 W2 + b2)
#   p  = h2 W3 + b3;  c = S(p)  (per-agent sparse linear map, 3072->1024)
#   s  = ||c||^2 ; w = W(256*s) ; u* = -sqrt(256*w/s) * c
#
# Host-side folds:
#   - S is linear: c = h2a @ W3a with h2a = [h2; 1], W3a = [[W3 S],[b3 S]].
#   - s = h2a^T (W3a W3a^T) h2a = h2a^T Ga h2a with the 101x101 Gram
#     matrix Ga precomputed on host.  No elementwise squares of c on
#     device, and k is ready BEFORE mm3 so the -k scale fuses with the
#     PSUM->SBUF move.
#   - z is bf16-cast and transposed to [D, B] on host (contraction on
#     partitions: no on-chip transposes, half the HBM traffic).  DRAM
#     layout gives each partition a contiguous run per DMA so HWDGE
#     descriptors are 8KB (descriptor GENERATION ~5ns/desc is the
#     DMA-issue bottleneck, not bandwidth).
#
# Lambert-W needs no iteration for k's accuracy (dk/k = dw/2w ~ dw/20):
#   w = alpha*ln(256 s) + beta  with ln from the fp32-exponent bit trick
#   k = 16*sqrt(w/s): rsqrt seed via ACT exp of the same bit trick,
#   one Newton step on DVE.  Only {tanh, exp} ACT functions are used ->
#   a single ACT_TABLE_LOAD for the whole kernel.
#
# Engine layout: z loads on sync/HWDGE; weight loads + output stores on
# the otherwise-idle GpSimd SWDGE ring; dummy matmuls keep the PE's HAM
# clock gate warm across the DMA-wait and Lambert windows.
#
# Sharding: pure data parallelism, batch 8192 -> 8 shards of 1024 rows.
# Output is written bf16 (within tolerance) and upcast on host.

import numpy as np
import ml_dtypes
from contextlib import ExitStack

import concourse.bass as bass
import concourse.tile as tile
from concourse import bacc, mybir
from concourse.bass_utils import run_bass_kernel_spmd

F32 = mybir.dt.float32
I32 = mybir.dt.int32
BF16 = mybir.dt.bfloat16

N_CORES = 8
BATCH = 8192
B = BATCH // N_CORES      # 1024 batch rows per core
D = 3072                  # state dim
H = 100                   # hidden
HA = H + 1                # hidden + ones row
CD = 1024                 # control dim
NCH = D // 128            # 24 contraction chunks for mm1
# batch groups: a big streaming group and a small final group so the
# post-stream serial tail (tanh->mm2->tanh->gram->s->lambert->mm3) covers
# as few columns as possible
GROUPS = [(0, 512), (512, 256), (768, 256)]   # (col start, width)
NG = len(GROUPS)
ZB = 8                    # j-chunks per z DMA
MASS = 0.5

AF = mybir.ActivationFunctionType
ALU = mybir.AluOpType

LN2 = 0.6931471805599453
LN256 = 5.545177444479562
# ln(x) ~= LN2 * (float(bitcast_i32(x)) * 2^-23 - 126.94269504)
LNA = LN2 / (1 << 23)
LNB = -126.94269504 * LN2
# w0 = alpha*ln(256 s) + beta (fit of W(e^L)=L-lnL+lnL/L over L in [11,14])
W0_ALPHA = 0.9103
W0_BETA = -1.2024
W0_A = W0_ALPHA * LNA
W0_B = W0_ALPHA * (LNB + LN256) + W0_BETA
# rsqrt seed: y0 = exp(aif*(-0.5*LNA) + (-0.5*LNB)) ~= 1/sqrt(a)
RS_SCALE = -0.5 * LNA
RS_BIAS = -0.5 * LNB


def build_kernel():
    nc = bacc.Bacc(None, target_bir_lowering=False, enable_partition_id=False)

    # zt[p, cstart_g*NCH + j*Wg + f] = z[gstart_g + f, j*128 + p]
    zt_d = nc.declare_dram_parameter("zt", [128, NCH * B], BF16, isOutput=False)
    # w1x chunk 24: row0 = W1[0,:] (t weights), row1 = b1 (ones-row weights)
    w1x_d = nc.declare_dram_parameter("w1x", [128, (NCH + 1) * H], BF16, isOutput=False)
    # wall: [0:101, 0:100] = [W2; b2], [0:101, 128:229] = Ga,
    # [0:101, 256:1280] = W3a, [0:2, 1280:2304] = te2 ([t; ones]).
    # Padded to 128 partitions: DMAs with <128 partitions are NOT split
    # across the 16 SDMA engines and serialize on engine 0, straggling
    # every later z-completion semaphore by ~10us.
    wall_d = nc.declare_dram_parameter("wall", [128, 2304], BF16, isOutput=False)
    out_d = nc.declare_dram_parameter("out", [B, CD], BF16, isOutput=True)

    with ExitStack() as ctx:
        tc = ctx.enter_context(tile.TileContext(nc))

        const = ctx.enter_context(tc.tile_pool(name="const", bufs=1))
        zpool = ctx.enter_context(tc.tile_pool(name="zt", bufs=NG))
        hpool = ctx.enter_context(tc.tile_pool(name="hs", bufs=2))
        lwp = ctx.enter_context(tc.tile_pool(name="lw", bufs=1))
        opool = ctx.enter_context(tc.tile_pool(name="outs", bufs=3))
        h1_ps = ctx.enter_context(tc.tile_pool(name="h1p", bufs=1, space="PSUM"))
        hq_ps = ctx.enter_context(tc.tile_pool(name="hqp", bufs=2, space="PSUM"))
        c_ps = ctx.enter_context(tc.tile_pool(name="cp", bufs=4, space="PSUM"))
        s_ps = ctx.enter_context(tc.tile_pool(name="sp", bufs=1, space="PSUM"))

        # ---- t=0: warm the ACT table (tanh+exp set) and the PE HAM clock
        # under the DMA shadow; small on-chip constants.
        warm_in = const.tile([128, 1], F32, tag="warm_in")
        nc.vector.memset(warm_in[:], 0.0)
        warm_out = const.tile([128, 1], F32, tag="warm_out")
        nc.scalar.activation(warm_out[:], warm_in[:], AF.Tanh, bias=warm_in[:])
        wsrc = const.tile([128, 512], BF16, tag="wsrc")
        nc.vector.memset(wsrc[:], 0.0)
        wps = h1_ps.tile([128, 512], F32, tag="h1p", name="wps")
        for _ in range(8):
            nc.tensor.matmul(wps[:], wsrc[:, 0:128], wsrc[:], start=True, stop=True)
        ones_a = const.tile([HA, 1], BF16, tag="ones_a")
        nc.vector.memset(ones_a[:], 1.0)
        rsb = const.tile([128, 1], F32, tag="rsb")
        nc.vector.memset(rsb[:], RS_BIAS)

        # ---- all loads on ONE ring (sync/HWDGE), interleaved in need-order:
        # secondary rings (scalar HWDGE, gpsimd SWDGE) get starved by the
        # z stream, so weights ride the same FIFO just ahead of their use.
        w1s = const.tile([128, NCH + 1, H], BF16, tag="w1s")
        wall = const.tile([128, 2304], BF16, tag="wall")
        w2a = wall[0:HA, 0:H]
        ga = wall[0:HA, 128:128 + HA]
        te2 = wall[0:2, 1280:1280 + B]

        def load_w1x(part):
            cs, ce = [(0, 9), (9, 17), (17, 25)][part]
            nc.sync.dma_start(
                w1s[:, cs:ce, :],
                w1x_d[:, cs * H:ce * H].rearrange("p (c h) -> p c h", c=ce - cs),
            )

        zg = {g: zpool.tile([128, NCH, GROUPS[g][1]], BF16, tag=f"zg{g}",
                            name=f"zg{g}")
              for g in range(NG)}

        def load_z(g, jb):
            gs, w = GROUPS[g]
            c0 = NCH * gs + jb * ZB * w
            nc.sync.dma_start(
                zg[g][:, jb * ZB:(jb + 1) * ZB, :],
                zt_d[:, c0:c0 + ZB * w].rearrange("p (c f) -> p c f", c=ZB),
            )

        load_w1x(0)
        load_w1x(1)
        load_w1x(2)
        nc.sync.dma_start(wall[:], wall_d[:])
        for g in range(NG):
            for jb in range(NCH // ZB):
                load_z(g, jb)

        s_all = s_ps.tile([128, B // 128], F32, tag="s_all")
        SCOL = [GROUPS[g][0] // 128 for g in range(NG)]

        def lambert(g):
            """kneg = -16*sqrt(w/s) for group g's chunks of s_all."""
            cpg = GROUPS[g][1] // 128

            def lt(nm, dt=F32):
                return lwp.tile([128, cpg], dt, tag=f"{nm}{g}", name=f"{nm}{g}")

            sv = s_all[:, SCOL[g]:SCOL[g] + cpg]
            sg = lt("sg")
            nc.vector.tensor_scalar_max(sg[:], sv, 1e-20)
            sif = lt("sif")
            nc.vector.tensor_copy(sif[:], sg[:].bitcast(I32))
            w = lt("w")
            nc.vector.tensor_scalar(w[:], sif[:], W0_A, W0_B, ALU.mult, ALU.add)
            rs = lt("rs")
            nc.vector.reciprocal_approx_fast(rs[:], sg[:])
            a = lt("a")
            nc.vector.tensor_mul(a[:], w[:], rs[:])
            aif = lt("aif")
            nc.vector.tensor_copy(aif[:], a[:].bitcast(I32))
            y0 = lt("y0")
            nc.scalar.activation(y0[:], aif[:], AF.Exp, bias=rsb[:], scale=RS_SCALE)
            # Newton step with -16 folded in: kneg = a*y0*(8*a*y0^2 - 24)
            yy = lt("yy")
            nc.vector.tensor_mul(yy[:], y0[:], y0[:])
            nc.vector.tensor_mul(yy[:], a[:], yy[:])
            nc.vector.tensor_scalar(yy[:], yy[:], 8.0, -24.0, ALU.mult, ALU.add)
            nc.vector.tensor_mul(y0[:], y0[:], yy[:])
            kneg = lt("kneg")
            nc.vector.tensor_mul(kneg[:], a[:], y0[:])
            return kneg

        for g in range(NG):
            cs, W = GROUPS[g]
            cpg = W // 128
            # mm1: h1p[h, b] = W1[0,h]*t[b] + b1[h] + sum_d W1[1+d,h] zT[d,b]
            h1p = h1_ps.tile([H, W], F32, tag="h1p", name="h1p")
            nc.tensor.matmul(
                h1p[:], w1s[0:2, NCH, :], wall[0:2, 1280 + cs:1280 + cs + W],
                start=True, stop=False,
            )
            for j in range(NCH):
                nc.tensor.matmul(
                    h1p[:], w1s[:, j, :], zg[g][:, j, :],
                    start=False, stop=(j == NCH - 1),
                )
            # tanh -> h1a with ones row at partition 100 (feeds b2 via w2a)
            h1a = hpool.tile([128, W], BF16, tag="h1s", name="h1a")
            nc.vector.memset(h1a[96:128, :], 1.0)
            nc.scalar.activation(h1a[0:H, :], h1p[:], AF.Tanh, bias=warm_in[0:H, :])
            # mm2 + tanh -> h2a with ones row at partition 100
            h2p = hq_ps.tile([H, W], F32, tag="hqp", name="h2p")
            nc.tensor.matmul(h2p[:], w2a, h1a[0:HA, :], start=True, stop=True)
            h2a = hpool.tile([128, W], BF16, tag="h2a", name="h2a")
            nc.vector.memset(h2a[96:128, :], 1.0)
            nc.scalar.activation(h2a[0:H, :], h2p[:], AF.Tanh, bias=warm_in[0:H, :])
            # Gram: q = Ga @ h2a ; sel = h2a*q ; s = colsum(sel) via PE
            qp = hq_ps.tile([HA, W], F32, tag="hqp", name="qp")
            nc.tensor.matmul(qp[:], ga, h2a[0:HA, :], start=True, stop=True)
            sel = hpool.tile([HA, W], BF16, tag="sel", name="sel")
            nc.vector.tensor_mul(sel[:], h2a[0:HA, :], qp[:])
            for i in range(cpg):
                nc.tensor.matmul(
                    s_all[:, SCOL[g] + i:SCOL[g] + i + 1],
                    sel[:, i * 128:(i + 1) * 128], ones_a[:],
                    start=True, stop=True,
                )
            kneg = lambert(g)
            # mm3 per 128-chunk; -k scale fused into the PSUM->SBUF move,
            # halves alternating DVE / ACT so neither engine is the tail
            for i in range(cpg):
                ot = opool.tile([128, CD], BF16, tag="ot", name="ot")
                for hf in range(2):
                    cp = c_ps.tile([128, 512], F32, tag="cp", name="cp")
                    nc.tensor.matmul(
                        cp[:], h2a[0:HA, i * 128:(i + 1) * 128],
                        wall[0:HA, 256 + hf * 512:256 + (hf + 1) * 512],
                        start=True, stop=True,
                    )
                    if hf == 0:
                        nc.vector.tensor_scalar(
                            ot[:, hf * 512:(hf + 1) * 512], cp[:],
                            kneg[:, i:i + 1], None, ALU.mult,
                        )
                    else:
                        nc.scalar.activation(
                            ot[:, hf * 512:(hf + 1) * 512], cp[:],
                            AF.Copy, bias=0.0, scale=kneg[:, i:i + 1],
                        )
                bt = cs // 128 + i
                # SWDGE: keeps stores off the sync HWDGE ring (descriptor-gen
                # bound) and off the scalar ring (would block later ACT ops)
                nc.gpsimd.dma_start(out_d[bt * 128:(bt + 1) * 128, :], ot[:])

    nc.compile()
    return nc


def host_prep(z, t, W1, b1, W2, b2, W3, b3):
    """Host-side weight folds, bf16 casts, z transpose, per-core shards."""
    f = np.float32
    bf = ml_dtypes.bfloat16
    z = np.asarray(z, f)
    t = np.asarray(t, f)
    W1 = np.asarray(W1, f)
    b1 = np.asarray(b1, f)
    W2 = np.asarray(W2, f)
    b2 = np.asarray(b2, f)
    W3 = np.asarray(W3, f)
    b3 = np.asarray(b3, f)

    # mm1 stationary chunks: w1x[p, j*H + h] = W1[1 + j*128 + p, h];
    # chunk NCH rows 0/1 carry W1[0,:] (t weights) and b1 (ones-row weights)
    w1x = np.zeros((128, NCH + 1, H), np.float32)
    w1x[:, :NCH, :] = W1[1:, :].reshape(NCH, 128, H).transpose(1, 0, 2)
    w1x[0, NCH, :] = W1[0, :]
    w1x[1, NCH, :] = b1
    w1x = np.ascontiguousarray(w1x.reshape(128, (NCH + 1) * H)).astype(bf)

    # fold the p -> c map into W3 / b3, then the ones-row bias fold
    W3r = W3.reshape(H, CD // 4, 12)
    W3S = np.empty((H, CD // 4, 4), f)
    W3S[..., 0] = (W3r[..., 6] + W3r[..., 7] + W3r[..., 8]) / MASS
    W3S[..., 1] = W3r[..., 9]
    W3S[..., 2] = W3r[..., 10]
    W3S[..., 3] = W3r[..., 11]
    b3r = b3.reshape(CD // 4, 12)
    b3S = np.empty((CD // 4, 4), f)
    b3S[..., 0] = (b3r[..., 6] + b3r[..., 7] + b3r[..., 8]) / MASS
    b3S[..., 1] = b3r[..., 9]
    b3S[..., 2] = b3r[..., 10]
    b3S[..., 3] = b3r[..., 11]
    w3a = np.concatenate([W3S.reshape(H, CD), b3S.reshape(1, CD)], axis=0)
    ga = w3a @ w3a.T
    wall0 = np.zeros((128, 2304), f)
    wall0[0:H, 0:H] = W2
    wall0[H, 0:H] = b2
    wall0[0:HA, 128:128 + HA] = ga
    wall0[0:HA, 256:256 + CD] = w3a
    wall0[1, 1280:1280 + B] = 1.0

    zb = z.astype(bf)

    in_maps = []
    for c in range(N_CORES):
        sl = slice(c * B, (c + 1) * B)
        # zt[p, NCH*gs + j*W + f] = z[c*B + gs + f, j*128 + p]
        zT = zb[sl].T.reshape(NCH, 128, B)
        zt = np.empty((128, NCH * B), zb.dtype)
        for gs, W in GROUPS:
            blk = zT[:, :, gs:gs + W].transpose(1, 0, 2).reshape(128, NCH * W)
            zt[:, NCH * gs:NCH * (gs + W)] = blk
        zt = np.ascontiguousarray(zt)
        wall0[0, 1280:1280 + B] = t[sl, 0]
        in_maps.append({
            "zt": zt,
            "w1x": w1x,
            "wall": np.ascontiguousarray(wall0).astype(bf),
        })
    return in_maps


_NC_CACHE = None


def _get_nc():
    global _NC_CACHE
    if _NC_CACHE is None:
        _NC_CACHE = build_kernel()
    return _NC_CACHE


def run(inputs, trace=False):
    """Returns (full_output, BassKernelResults)."""
    nc = _get_nc()
    in_maps = host_prep(**inputs)
    res = run_bass_kernel_spmd(
        nc, in_maps, list(range(N_CORES)), trace=trace,
    )
    out = np.concatenate(
        [np.asarray(r["out"]).astype(np.float32) for r in res.results], axis=0
    )
    return out, res


def kernel(**inputs):
    out, _ = run(inputs)
    return out
